# revision 21
# baseline (speedup 1.0000x reference)
"""DotAttention Trainium2 Bass kernel.

out[b] = softmax(Q[b] @ K[b]^T, axis=-1) @ K[b]
  Q: [16, 1024, 4096] f32, K: [16, 2048, 4096] f32 -> out [16, 1024, 4096] f32

Sharding: batch dim across 8 NeuronCores (2 batches/core), fully local.

Host-side prep (inside kernel(), per core): cast to fp16 and lay the
operands out in matmul-native form so the device does zero transposes or
casts of Q/K:
  - QT  [b, 128p, 1024q, 32dc] fp16 : QT[b,p,q,dc] = Q[b,q,dc*128+p]
  - KT  [b, 128p, 32dc, 2048k] fp16 : KT[b,p,dc,k] = K[b,k,dc*128+p]
  - KN8 [b, 16kc, 128p, 2, 4096] fp8e4m3: plane 0 = fp8(K), plane 1 =
    fp8(K - fp8(K)) (the quantization residual), rows kc*128+p.

Device per batch:
  1. Logits A = Q K^T per k-quarter (512 keys), fp16 matmuls, fp32 PSUM.
     Online softmax: per-quarter negated max m_q, e = exp(a - m_q) fp16,
     accumulated sums.
  2. Merge: global max, f_q = exp(m_q - m) rescale of E, r = 1/sum.
  3. E rows xbar-transposed per q-tile, cast fp8.
  4. C = E8^T.T @ (K8 + Klo8): fp8 DoubleRow matmuls, each contracting
     (E8*K8 + E8*Klo8) via a stride-0-broadcast lhsT against the
     interleaved hi/lo K planes -- K at ~fp16 accuracy, 2x rate.  Four
     passes over d-quarters with double-buffered K tiles in the slots Q
     vacates after the logits phase.  Normalization by r folds into the
     PSUM->SBUF copy (ACT scale); output stored fp16.

Measured end-to-end relative error ~0.004 (gate 2e-2).
"""

import numpy as np
import ml_dtypes

import concourse.bass as bass
import concourse.bacc as bacc
import concourse.mybir as mybir
import concourse.tile as tile
from concourse.bass_utils import run_bass_kernel_spmd

P = 128
N_CORES = 8
B_FULL, LQ, LK, D = 16, 1024, 2048, 4096
B_PER_CORE = B_FULL // N_CORES  # 2

F16 = mybir.dt.float16
F32 = mybir.dt.float32
F8 = mybir.dt.float8e4
AX = mybir.AxisListType
AF = mybir.ActivationFunctionType
DR = mybir.MatmulPerfMode.DoubleRow

E4M3 = ml_dtypes.float8_e4m3


def build_program(b_per_core=B_PER_CORE, lq=LQ, lk=LK, d=D):
    nqt = lq // P          # 8 q-tiles
    nkc = lk // P          # 16 k-chunks
    nqtr = 4               # k-quarters for online softmax
    qtr_k = lk // nqtr     # 512 keys per quarter
    dc_n = d // P          # 32 d-chunks
    nqd = 4                # d-quarters for the second matmul
    qd_d = d // nqd        # 1024

    nc = bacc.Bacc("TRN2", target_bir_lowering=False, debug=False, num_swdge_queues=4)
    qt_dram = nc.dram_tensor("qt", [b_per_core, P, lq, dc_n], F16, kind="ExternalInput").ap()
    kt_dram = nc.dram_tensor("kt", [b_per_core, P, dc_n, lk], F16, kind="ExternalInput").ap()
    kn_dram = nc.dram_tensor("kn8", [b_per_core, P, nkc, 2, d], F8, kind="ExternalInput").ap()
    o_dram = nc.dram_tensor("out", [b_per_core, lq, d], F16, kind="ExternalOutput").ap()

    with tile.TileContext(nc) as tc:
        with (
            # 2x 32KB: Q lo/hi halves during logits, then K8/Klo8 d-quarters
            tc.tile_pool(name="qkn", bufs=2) as qkn,
            # 2x 32KB: K^T quarters (rotating)
            tc.tile_pool(name="ktq", bufs=2) as ktq_pool,
            # 32KB: unscaled/rescaled E [qt, k] fp16
            tc.tile_pool(name="epool", bufs=1) as epool,
            # 16KB: E^T fp8 for the whole batch [qt, kc, q]
            tc.tile_pool(name="e8t", bufs=1) as e8tp,
            # 4KB: fp16 E^T staging per q-tile
            tc.tile_pool(name="ett", bufs=2) as ettp,
            # 2KB: fp16 output staging
            tc.tile_pool(name="cout", bufs=6) as coutp,
            tc.tile_pool(name="stats", bufs=2) as stats,
            tc.tile_pool(name="psumL", bufs=2, space="PSUM") as psumL,
            tc.tile_pool(name="psumC", bufs=3, space="PSUM") as psumC,
        ):
            for b in range(b_per_core):
                # ---- loads for this batch (SP queue, slot waits pace them) ----
                ktq = []
                k0 = ktq_pool.tile([P, dc_n, qtr_k], F16, tag="ktq", name=f"ktq_{b}_0")
                q_lo = qkn.tile([P, lq // 2, dc_n], F16, tag="qkn", name=f"qlo_{b}")
                q_hi = qkn.tile([P, lq // 2, dc_n], F16, tag="qkn", name=f"qhi_{b}")
                ktq.append(k0)
                if b == 0:
                    # cold start: interleave fine K^T-quarter-0 / Q pieces so
                    # the first logits matmuls start after ~2 small transfers
                    k_pieces = [(0, 4), (4, 12), (12, 20), (20, 28), (28, 32)]
                    q_pieces = [(0, 128), (128, 256), (256, 384), (384, 512), None]
                    for kp, qp in zip(k_pieces, q_pieces):
                        nc.sync.dma_start(
                            out=k0[:, kp[0] : kp[1], :],
                            in_=kt_dram[b, :, kp[0] : kp[1], 0:qtr_k],
                        )
                        if qp is not None:
                            nc.sync.dma_start(
                                out=q_lo[:, qp[0] : qp[1], :],
                                in_=qt_dram[b, :, qp[0] : qp[1], :],
                            )
                else:
                    nc.sync.dma_start(out=k0[:], in_=kt_dram[b, :, :, 0:qtr_k])
                    nc.sync.dma_start(out=q_lo[:], in_=qt_dram[b, :, : lq // 2, :])
                # q_hi is needed a quarter-length before K^T quarter 1
                nc.sync.dma_start(out=q_hi[:], in_=qt_dram[b, :, lq // 2 :, :])
                k1 = ktq_pool.tile([P, dc_n, qtr_k], F16, tag="ktq", name=f"ktq_{b}_1")
                nc.sync.dma_start(out=k1[:], in_=kt_dram[b, :, :, qtr_k : 2 * qtr_k])
                ktq.append(k1)

                for q4 in (2, 3):
                    kq = ktq_pool.tile([P, dc_n, qtr_k], F16, tag="ktq",
                                       name=f"ktq_{b}_{q4}")
                    nc.sync.dma_start(out=kq[:], in_=kt_dram[b, :, :, q4 * qtr_k : (q4 + 1) * qtr_k])
                    ktq.append(kq)

                def q_lhsT(qt, dc):
                    t = q_lo if qt < 4 else q_hi
                    i = qt % 4
                    return t[:, i * P : (i + 1) * P, dc]

                # ---- per-batch softmax stats ----
                M = stats.tile([P, nqt, nqtr], F32, tag="m", name=f"M_{b}")
                S = stats.tile([P, nqt, nqtr], F32, tag="s", name=f"S_{b}")
                F = stats.tile([P, nqt, nqtr], F32, tag="f", name=f"F_{b}")
                R = stats.tile([P, nqt], F32, tag="r", name=f"R_{b}")
                E = epool.tile([P, nqt, lk], F16, tag="e", name=f"E_{b}")

                # ---- logits + per-quarter online softmax ----
                # During the last quarter each finished q-tile immediately
                # runs its merge + E^T transpose + fp8 cast, and the first
                # K8/Klo8 d-quarter loads slot in between, so the second
                # matmul starts with zero PE idle.
                e8t = e8tp.tile([P, nqt, nkc, P], F8, tag="e8t", name=f"e8t_{b}")
                knq_tiles = {}

                def merge_chain(qt):
                    negm = stats.tile([P, 1], F32, tag="negm", name=f"negm_{b}_{qt}")
                    nc.vector.tensor_reduce(
                        negm, M[:, qt, :], axis=AX.X, op=mybir.AluOpType.min
                    )
                    nc.scalar.activation(
                        F[:, qt, :], M[:, qt, :], AF.Exp, bias=negm, scale=-1.0
                    )
                    fs = stats.tile([P, nqtr], F32, tag="fs", name=f"fs_{b}_{qt}")
                    nc.vector.tensor_mul(fs, F[:, qt, :], S[:, qt, :])
                    sg = stats.tile([P, 1], F32, tag="sg", name=f"sg_{b}_{qt}")
                    nc.vector.reduce_sum(sg, fs, axis=AX.X)
                    nc.vector.reciprocal(R[:, qt : qt + 1], sg)
                    for q4 in range(nqtr):
                        sl = E[:, qt, q4 * qtr_k : (q4 + 1) * qtr_k]
                        nc.vector.tensor_scalar_mul(sl, sl, F[:, qt, q4 : q4 + 1])
                    ett = ettp.tile([P, nkc, P], F16, tag="ett", name=f"ett_{b}_{qt}")
                    nc.sync.dma_start_transpose(ett, E[:, qt, :])
                    # cast on the otherwise-idle GPSIMD so the DVE FIFO never
                    # waits on the transpose DMA
                    nc.gpsimd.tensor_copy(e8t[:, qt, :, :], ett)

                def load_knq(qd, pool):
                    knq = pool.tile([P, nkc, 2, qd_d], F8,
                                    tag="ktq" if pool is ktq_pool else "qkn",
                                    name=f"knq_{b}_{qd}")
                    nc.sync.dma_start(
                        out=knq[:],
                        in_=kn_dram[b, :, :, :, qd * qd_d : (qd + 1) * qd_d],
                    )
                    knq_tiles[qd] = knq

                for q4 in range(nqtr):
                    if q4 == nqtr - 1:
                        # K8/Klo8 d-quarter 0 goes in the ktq buffer that
                        # quarter 2 just vacated: its load fully overlaps the
                        # last logits quarter
                        load_knq(0, ktq_pool)
                    for qt in range(nqt):
                        aps = psumL.tile([P, qtr_k], F32, tag="psL",
                                         name=f"aps_{b}_{q4}_{qt}")
                        for dc in range(dc_n):
                            nc.tensor.matmul(
                                aps,
                                q_lhsT(qt, dc),
                                ktq[q4][:, dc, :],
                                start=(dc == 0),
                                stop=(dc == dc_n - 1),
                            )
                        nc.vector.reduce_max(
                            M[:, qt, q4 : q4 + 1], aps, axis=AX.X, negate=True
                        )
                        nc.scalar.activation(
                            E[:, qt, q4 * qtr_k : (q4 + 1) * qtr_k], aps, AF.Exp,
                            bias=M[:, qt, q4 : q4 + 1], scale=1.0,
                            accum_out=S[:, qt, q4 : q4 + 1],
                        )
                        if q4 == nqtr - 1:
                            merge_chain(qt)
                            if qt == 3:
                                load_knq(1, qkn)  # q_lo slot frees here

                # ---- second matmul: 4 passes over d-quarters ----
                for qd in range(nqd):
                    if qd not in knq_tiles:
                        load_knq(qd, qkn)
                    knq = knq_tiles[qd]
                    for qt in range(nqt):
                        cps = psumC.tile([P, qd_d], F32, tag="psC",
                                         name=f"cps_{b}_{qd}_{qt}")
                        last_tile = (
                            b == b_per_core - 1 and qd == nqd - 1 and qt == nqt - 1
                        )
                        nbs = qd_d // 512
                        nb_groups = (
                            [[nb] for nb in range(nbs)] if last_tile
                            else [list(range(nbs))]
                        )
                        for grp in nb_groups:
                            for kc in range(nkc):
                                for nb in grp:
                                    nc.tensor.matmul(
                                        cps[:, nb * 512 : (nb + 1) * 512],
                                        e8t[:, qt, kc : kc + 1, :].broadcast_to([P, 2, P]),
                                        knq[:, kc, :, nb * 512 : (nb + 1) * 512],
                                        start=(kc == 0),
                                        stop=(kc == nkc - 1),
                                        perf_mode=DR,
                                    )
                            if last_tile:
                                # drain the tail in halves so the final store
                                # overlaps the last accumulation group
                                for nb in grp:
                                    c_out = coutp.tile([P, 512], F16, tag="co",
                                                       name=f"co_l_{nb}")
                                    nc.scalar.mul(
                                        c_out, cps[:, nb * 512 : (nb + 1) * 512],
                                        R[:, qt : qt + 1],
                                    )
                                    nc.scalar.dma_start(
                                        out=o_dram[
                                            b, qt * P : (qt + 1) * P,
                                            qd * qd_d + nb * 512 : qd * qd_d + (nb + 1) * 512,
                                        ],
                                        in_=c_out,
                                    )
                        if not last_tile:
                            c_out = coutp.tile([P, qd_d], F16, tag="co",
                                               name=f"co_{b}_{qd}_{qt}")
                            nc.scalar.mul(c_out, cps, R[:, qt : qt + 1])
                            # store right behind the copy on the ACT queue;
                            # Pool stays dedicated to the fp8 casts
                            nc.scalar.dma_start(
                                out=o_dram[b, qt * P : (qt + 1) * P, qd * qd_d : (qd + 1) * qd_d],
                                in_=c_out,
                            )
    nc.compile()
    return nc


_PROGRAM = None


def _get_program():
    global _PROGRAM
    if _PROGRAM is None:
        _PROGRAM = build_program()
    return _PROGRAM


LAST_RESULTS = None  # BassKernelResults of the most recent kernel() call


def _prep_core(qb: np.ndarray, kb: np.ndarray):
    """Host-side layout prep for one core's batch slice (see module doc)."""
    b = qb.shape[0]
    q16 = qb.astype(np.float16)
    k16 = kb.astype(np.float16)
    qt = np.ascontiguousarray(
        q16.reshape(b, LQ, D // P, P).transpose(0, 3, 1, 2)
    )
    kt = np.ascontiguousarray(
        k16.reshape(b, LK, D // P, P).transpose(0, 3, 2, 1)
    )
    k8 = kb.astype(E4M3)
    klo8 = (kb - k8.astype(np.float32)).astype(E4M3)
    # [b, kc, p, 2, d] -> [b, p, kc, 2, d] so the partition dim leads the
    # fused on-device DMA
    kn8 = np.ascontiguousarray(
        np.stack(
            [
                np.asarray(k8).reshape(b, LK // P, P, D),
                np.asarray(klo8).reshape(b, LK // P, P, D),
            ],
            axis=3,
        ).transpose(0, 2, 1, 3, 4)
    )
    return {"qt": qt, "kt": kt, "kn8": kn8}


def kernel(query: np.ndarray, key: np.ndarray) -> np.ndarray:
    global LAST_RESULTS
    query = np.ascontiguousarray(query, dtype=np.float32)
    key = np.ascontiguousarray(key, dtype=np.float32)
    assert query.shape == (B_FULL, LQ, D), query.shape
    assert key.shape == (B_FULL, LK, D), key.shape

    nc = _get_program()
    in_maps = [
        _prep_core(
            query[i * B_PER_CORE : (i + 1) * B_PER_CORE],
            key[i * B_PER_CORE : (i + 1) * B_PER_CORE],
        )
        for i in range(N_CORES)
    ]
    try:
        res = run_bass_kernel_spmd(nc, in_maps, core_ids=list(range(N_CORES)))
    except Exception:
        # one retry: absorbs transient device wedges (NRT_EXEC_UNIT_*)
        res = run_bass_kernel_spmd(nc, in_maps, core_ids=list(range(N_CORES)))
    LAST_RESULTS = res
    out = np.concatenate([r["out"] for r in res.results], axis=0)
    return np.ascontiguousarray(out.astype(np.float32))


# revision 32
# speedup vs baseline: 1.1333x; 1.1333x over previous
"""DotAttention Trainium2 Bass kernel.

out[b] = softmax(Q[b] @ K[b]^T, axis=-1) @ K[b]
  Q: [16, 1024, 4096] f32, K: [16, 2048, 4096] f32 -> out [16, 1024, 4096] f32

Sharding: batch dim across 8 NeuronCores (2 batches/core), fully local.

Host-side prep (inside kernel(), per core): cast to fp16 and lay the
operands out in matmul-native form so the device does zero transposes or
casts of Q/K:
  - QT  [b, 128p, 1024q, 32dc] fp16 : QT[b,p,q,dc] = Q[b,q,dc*128+p]
  - KT  [b, 128p, 32dc, 2048k] fp16 : KT[b,p,dc,k] = K[b,k,dc*128+p]
  - KN8 [b, 16kc, 128p, 2, 4096] fp8e4m3: plane 0 = fp8(K), plane 1 =
    fp8(K - fp8(K)) (the quantization residual), rows kc*128+p.

Device per batch:
  1. Logits A = Q K^T per k-quarter (512 keys), fp16 matmuls, fp32 PSUM.
     Online softmax: per-quarter negated max m_q, e = exp(a - m_q) fp16,
     accumulated sums.
  2. Merge: global max, f_q = exp(m_q - m) rescale of E, r = 1/sum.
  3. E rows xbar-transposed per q-tile, cast fp8.
  4. C = E8^T.T @ (K8 + Klo8): fp8 DoubleRow matmuls, each contracting
     (E8*K8 + E8*Klo8) via a stride-0-broadcast lhsT against the
     interleaved hi/lo K planes -- K at ~fp16 accuracy, 2x rate.  Four
     passes over d-quarters with double-buffered K tiles in the slots Q
     vacates after the logits phase.  Normalization by r folds into the
     PSUM->SBUF copy (ACT scale); output stored fp16.

Measured end-to-end relative error ~0.004 (gate 2e-2).
"""

import numpy as np
import ml_dtypes

import concourse.bass as bass
import concourse.bacc as bacc
import concourse.mybir as mybir
import concourse.tile as tile
from concourse.bass_utils import run_bass_kernel_spmd

P = 128
N_CORES = 8
B_FULL, LQ, LK, D = 16, 1024, 2048, 4096
B_PER_CORE = B_FULL // N_CORES  # 2

F16 = mybir.dt.float16
F32 = mybir.dt.float32
F8 = mybir.dt.float8e4
AX = mybir.AxisListType
AF = mybir.ActivationFunctionType
DR = mybir.MatmulPerfMode.DoubleRow

E4M3 = ml_dtypes.float8_e4m3


def build_program(b_per_core=B_PER_CORE, lq=LQ, lk=LK, d=D):
    nqt = lq // P          # 8 q-tiles
    nkc = lk // P          # 16 k-chunks
    nqtr = 4               # k-quarters for online softmax
    qtr_k = lk // nqtr     # 512 keys per quarter
    dc_n = d // P          # 32 d-chunks
    nqd = 4                # d-quarters for the second matmul
    qd_d = d // nqd        # 1024

    nc = bacc.Bacc("TRN2", target_bir_lowering=False, debug=False, num_swdge_queues=4)
    # Q/K^T as fp8 hi/lo plane pairs (same bytes as fp16): logits are
    # computed as (Qh+Ql)@Kh + Qh@Kl via DoubleRow, dropping only Ql@Kl.
    # Q is q-tile-major so per-q-tile loads stay contiguous.
    qt_dram = nc.dram_tensor(
        "qt8", [b_per_core, P, lq // P, dc_n, 2, P], F8, kind="ExternalInput"
    ).ap()
    kt_dram = nc.dram_tensor(
        "kt8", [b_per_core, P, dc_n, 2, lk], F8, kind="ExternalInput"
    ).ap()
    kn_dram = nc.dram_tensor("kn8", [b_per_core, P, nkc, 2, d], F8, kind="ExternalInput").ap()
    o_dram = nc.dram_tensor("out", [b_per_core, lq, d], F16, kind="ExternalOutput").ap()

    with tile.TileContext(nc) as tc:
        with (
            # 2x 32KB: Q lo/hi halves during logits, then K8/Klo8 d-quarters
            tc.tile_pool(name="qkn", bufs=2) as qkn,
            # 2x 32KB: K^T quarters (rotating)
            tc.tile_pool(name="ktq", bufs=2) as ktq_pool,
            # 32KB: unscaled/rescaled E [qt, k] fp16
            tc.tile_pool(name="epool", bufs=1) as epool,
            # 16KB: E^T fp8 for the whole batch [qt, kc, q]
            tc.tile_pool(name="e8t", bufs=1) as e8tp,
            # 4KB: fp16 E^T staging per q-tile
            tc.tile_pool(name="ett", bufs=2) as ettp,
            # 2KB: fp16 output staging
            tc.tile_pool(name="cout", bufs=6) as coutp,
            tc.tile_pool(name="stats", bufs=2) as stats,
            tc.tile_pool(name="psumL", bufs=2, space="PSUM") as psumL,
            tc.tile_pool(name="psumC", bufs=3, space="PSUM") as psumC,
        ):
            for b in range(b_per_core):
                # ---- loads for this batch (SP queue, slot waits pace them) ----
                ktq = []
                k0 = ktq_pool.tile([P, dc_n, 2, qtr_k], F8, tag="ktq", name=f"ktq_{b}_0")
                q_lo = qkn.tile([P, 4, dc_n, 2, P], F8, tag="qkn", name=f"qlo_{b}")
                q_hi = qkn.tile([P, 4, dc_n, 2, P], F8, tag="qkn", name=f"qhi_{b}")
                ktq.append(k0)
                if b == 0:
                    # cold start: interleave fine K^T-quarter-0 / Q pieces so
                    # the first logits matmuls start after ~2 small transfers
                    k_pieces = [(0, 4), (4, 12), (12, 20), (20, 28), (28, 32)]
                    q_pieces = [0, 1, 2, 3, None]
                    for kp, qp in zip(k_pieces, q_pieces):
                        nc.sync.dma_start(
                            out=k0[:, kp[0] : kp[1], :, :],
                            in_=kt_dram[b, :, kp[0] : kp[1], :, 0:qtr_k],
                        )
                        if qp is not None:
                            nc.sync.dma_start(
                                out=q_lo[:, qp, :, :, :],
                                in_=qt_dram[b, :, qp, :, :, :],
                            )
                if b != 0:
                    nc.sync.dma_start(out=k0[:], in_=kt_dram[b, :, :, :, 0:qtr_k])
                    nc.sync.dma_start(out=q_lo[:], in_=qt_dram[b, :, 0:4, :, :, :])
                # q_hi is needed a quarter-length before K^T quarter 1
                nc.sync.dma_start(out=q_hi[:], in_=qt_dram[b, :, 4:8, :, :, :])
                k1 = ktq_pool.tile([P, dc_n, 2, qtr_k], F8, tag="ktq", name=f"ktq_{b}_1")
                nc.sync.dma_start(out=k1[:], in_=kt_dram[b, :, :, :, qtr_k : 2 * qtr_k])
                ktq.append(k1)

                for q4 in (2, 3):
                    kq = ktq_pool.tile([P, dc_n, 2, qtr_k], F8, tag="ktq",
                                       name=f"ktq_{b}_{q4}")
                    nc.sync.dma_start(
                        out=kq[:], in_=kt_dram[b, :, :, :, q4 * qtr_k : (q4 + 1) * qtr_k]
                    )
                    ktq.append(kq)

                def q_tile(qt):
                    return (q_lo if qt < 4 else q_hi), qt % 4

                # ---- per-batch softmax stats ----
                M = stats.tile([P, nqt, nqtr], F32, tag="m", name=f"M_{b}")
                S = stats.tile([P, nqt, nqtr], F32, tag="s", name=f"S_{b}")
                F = stats.tile([P, nqt, nqtr], F32, tag="f", name=f"F_{b}")
                R = stats.tile([P, nqt], F32, tag="r", name=f"R_{b}")
                E = epool.tile([P, nqt, lk], F16, tag="e", name=f"E_{b}")

                # ---- logits + per-quarter online softmax ----
                # During the last quarter each finished q-tile immediately
                # runs its merge + E^T transpose + fp8 cast, and the first
                # K8/Klo8 d-quarter loads slot in between, so the second
                # matmul starts with zero PE idle.
                e8t = e8tp.tile([P, nqt, nkc, P], F8, tag="e8t", name=f"e8t_{b}")
                knq_tiles = {}

                def merge_chain(qt):
                    negm = stats.tile([P, 1], F32, tag="negm", name=f"negm_{b}_{qt}")
                    nc.vector.tensor_reduce(
                        negm, M[:, qt, :], axis=AX.X, op=mybir.AluOpType.min
                    )
                    nc.scalar.activation(
                        F[:, qt, :], M[:, qt, :], AF.Exp, bias=negm, scale=-1.0
                    )
                    fs = stats.tile([P, nqtr], F32, tag="fs", name=f"fs_{b}_{qt}")
                    nc.vector.tensor_mul(fs, F[:, qt, :], S[:, qt, :])
                    sg = stats.tile([P, 1], F32, tag="sg", name=f"sg_{b}_{qt}")
                    nc.vector.reduce_sum(sg, fs, axis=AX.X)
                    nc.vector.reciprocal(R[:, qt : qt + 1], sg)
                    for q4 in range(nqtr):
                        sl = E[:, qt, q4 * qtr_k : (q4 + 1) * qtr_k]
                        nc.vector.tensor_scalar_mul(sl, sl, F[:, qt, q4 : q4 + 1])
                    ett = ettp.tile([P, nkc, P], F16, tag="ett", name=f"ett_{b}_{qt}")
                    # halves: the second matmul's first k-chunks only wait on
                    # the first half of the transpose+cast chain
                    for h in range(2):
                        hk = nkc // 2
                        nc.sync.dma_start_transpose(
                            ett[:, h * hk : (h + 1) * hk, :],
                            E[:, qt, h * hk * P : (h + 1) * hk * P],
                        )
                        # cast on the otherwise-idle GPSIMD so the DVE FIFO
                        # never waits on the transpose DMA
                        nc.gpsimd.tensor_copy(
                            e8t[:, qt, h * hk : (h + 1) * hk, :],
                            ett[:, h * hk : (h + 1) * hk, :],
                        )

                def load_knq(qd, pool):
                    knq = pool.tile([P, nkc, 2, qd_d], F8,
                                    tag="ktq" if pool is ktq_pool else "qkn",
                                    name=f"knq_{b}_{qd}")
                    nc.sync.dma_start(
                        out=knq[:],
                        in_=kn_dram[b, :, :, :, qd * qd_d : (qd + 1) * qd_d],
                    )
                    knq_tiles[qd] = knq

                for q4 in range(nqtr):
                    if q4 == nqtr - 1:
                        # K8/Klo8 d-quarter 0 goes in the ktq buffer that
                        # quarter 2 just vacated: its load fully overlaps the
                        # last logits quarter
                        load_knq(0, ktq_pool)
                    for qt in range(nqt):
                        aps = psumL.tile([P, qtr_k], F32, tag="psL",
                                         name=f"aps_{b}_{q4}_{qt}")
                        qtile, qi = q_tile(qt)
                        for dc in range(dc_n):
                            # (Qh+Ql)[dc] @ Kh[dc]
                            nc.tensor.matmul(
                                aps,
                                qtile[:, qi, dc, :, :],
                                ktq[q4][:, dc, 0:1, :].broadcast_to([P, 2, qtr_k]),
                                start=(dc == 0),
                                stop=False,
                                perf_mode=DR,
                            )
                        for dcp in range(dc_n // 2):
                            # Qh[2p]@Kl[2p] + Qh[2p+1]@Kl[2p+1]
                            nc.tensor.matmul(
                                aps,
                                qtile[:, qi, 2 * dcp : 2 * dcp + 2, 0, :],
                                ktq[q4][:, 2 * dcp : 2 * dcp + 2, 1, :],
                                start=False,
                                stop=(dcp == dc_n // 2 - 1),
                                perf_mode=DR,
                            )
                        nc.vector.reduce_max(
                            M[:, qt, q4 : q4 + 1], aps, axis=AX.X, negate=True
                        )
                        nc.scalar.activation(
                            E[:, qt, q4 * qtr_k : (q4 + 1) * qtr_k], aps, AF.Exp,
                            bias=M[:, qt, q4 : q4 + 1], scale=1.0,
                            accum_out=S[:, qt, q4 : q4 + 1],
                        )
                        if q4 == nqtr - 1:
                            merge_chain(qt)

                # ---- second matmul: 4 passes over d-quarters ----
                # knq1 is issued only after every E^T transpose so its 11.6us
                # transfer never head-of-line blocks them; it is ready well
                # before pass 1 needs it
                load_knq(1, qkn)
                for qd in range(nqd):
                    if qd not in knq_tiles:
                        load_knq(qd, qkn)
                    knq = knq_tiles[qd]
                    for qt in range(nqt):
                        cps = psumC.tile([P, qd_d], F32, tag="psC",
                                         name=f"cps_{b}_{qd}_{qt}")
                        last_tile = (
                            b == b_per_core - 1 and qd == nqd - 1 and qt == nqt - 1
                        )
                        nbs = qd_d // 512
                        nb_groups = (
                            [[nb] for nb in range(nbs)] if last_tile
                            else [list(range(nbs))]
                        )
                        for grp in nb_groups:
                            for kc in range(nkc):
                                for nb in grp:
                                    nc.tensor.matmul(
                                        cps[:, nb * 512 : (nb + 1) * 512],
                                        e8t[:, qt, kc : kc + 1, :].broadcast_to([P, 2, P]),
                                        knq[:, kc, :, nb * 512 : (nb + 1) * 512],
                                        start=(kc == 0),
                                        stop=(kc == nkc - 1),
                                        perf_mode=DR,
                                    )
                            if last_tile:
                                # drain the tail in halves so the final store
                                # overlaps the last accumulation group
                                for nb in grp:
                                    c_out = coutp.tile([P, 512], F16, tag="co",
                                                       name=f"co_l_{nb}")
                                    nc.scalar.mul(
                                        c_out, cps[:, nb * 512 : (nb + 1) * 512],
                                        R[:, qt : qt + 1],
                                    )
                                    nc.scalar.dma_start(
                                        out=o_dram[
                                            b, qt * P : (qt + 1) * P,
                                            qd * qd_d + nb * 512 : qd * qd_d + (nb + 1) * 512,
                                        ],
                                        in_=c_out,
                                    )
                        if not last_tile:
                            c_out = coutp.tile([P, qd_d], F16, tag="co",
                                               name=f"co_{b}_{qd}_{qt}")
                            nc.scalar.mul(c_out, cps, R[:, qt : qt + 1])
                            # store right behind the copy on the ACT queue;
                            # Pool stays dedicated to the fp8 casts
                            nc.scalar.dma_start(
                                out=o_dram[b, qt * P : (qt + 1) * P, qd * qd_d : (qd + 1) * qd_d],
                                in_=c_out,
                            )
    nc.compile()
    return nc


_PROGRAM = None


def _get_program():
    global _PROGRAM
    if _PROGRAM is None:
        _PROGRAM = build_program()
    return _PROGRAM


LAST_RESULTS = None  # BassKernelResults of the most recent kernel() call


def _hilo(x: np.ndarray):
    hi = x.astype(E4M3)
    lo = (x - hi.astype(np.float32)).astype(E4M3)
    return np.asarray(hi), np.asarray(lo)


def _prep_core(qb: np.ndarray, kb: np.ndarray):
    """Host-side layout prep for one core's batch slice (see module doc)."""
    b = qb.shape[0]
    qh, ql = _hilo(qb)
    kh, kl = _hilo(kb)
    # qt8 [b, p, qt, dc, 2, qq]: plane j of (Qh, Ql) at [.., j, qq]
    # from Q[b, qt*128+qq, dc*128+p]
    qt8 = np.ascontiguousarray(
        np.stack(
            [
                qh.reshape(b, LQ // P, P, D // P, P),
                ql.reshape(b, LQ // P, P, D // P, P),
            ],
            axis=5,
        ).transpose(0, 4, 1, 3, 5, 2)
    )
    # kt8 [b, p, dc, 2, k] from K[b, k, dc*128+p]
    kt8 = np.ascontiguousarray(
        np.stack(
            [
                kh.reshape(b, LK, D // P, P),
                kl.reshape(b, LK, D // P, P),
            ],
            axis=4,
        ).transpose(0, 3, 2, 4, 1)
    )
    # kn8 [b, p, kc, 2, d]: natural-layout (K8, Klo8) planes for matmul 2
    kn8 = np.ascontiguousarray(
        np.stack(
            [
                kh.reshape(b, LK // P, P, D),
                kl.reshape(b, LK // P, P, D),
            ],
            axis=3,
        ).transpose(0, 2, 1, 3, 4)
    )
    return {"qt8": qt8, "kt8": kt8, "kn8": kn8}


def kernel(query: np.ndarray, key: np.ndarray) -> np.ndarray:
    global LAST_RESULTS
    query = np.ascontiguousarray(query, dtype=np.float32)
    key = np.ascontiguousarray(key, dtype=np.float32)
    assert query.shape == (B_FULL, LQ, D), query.shape
    assert key.shape == (B_FULL, LK, D), key.shape

    nc = _get_program()
    in_maps = [
        _prep_core(
            query[i * B_PER_CORE : (i + 1) * B_PER_CORE],
            key[i * B_PER_CORE : (i + 1) * B_PER_CORE],
        )
        for i in range(N_CORES)
    ]
    try:
        res = run_bass_kernel_spmd(nc, in_maps, core_ids=list(range(N_CORES)))
    except Exception:
        # one retry: absorbs transient device wedges (NRT_EXEC_UNIT_*)
        res = run_bass_kernel_spmd(nc, in_maps, core_ids=list(range(N_CORES)))
    LAST_RESULTS = res
    out = np.concatenate([r["out"] for r in res.results], axis=0)
    return np.ascontiguousarray(out.astype(np.float32))


# revision 36
# speedup vs baseline: 1.1425x; 1.0081x over previous
"""DotAttention Trainium2 Bass kernel.

out[b] = softmax(Q[b] @ K[b]^T, axis=-1) @ K[b]
  Q: [16, 1024, 4096] f32, K: [16, 2048, 4096] f32 -> out [16, 1024, 4096] f32

Sharding: batch dim across 8 NeuronCores (2 batches/core), fully local.

Host-side prep (inside kernel(), per core): cast to fp16 and lay the
operands out in matmul-native form so the device does zero transposes or
casts of Q/K:
  - QT  [b, 128p, 1024q, 32dc] fp16 : QT[b,p,q,dc] = Q[b,q,dc*128+p]
  - KT  [b, 128p, 32dc, 2048k] fp16 : KT[b,p,dc,k] = K[b,k,dc*128+p]
  - KN8 [b, 16kc, 128p, 2, 4096] fp8e4m3: plane 0 = fp8(K), plane 1 =
    fp8(K - fp8(K)) (the quantization residual), rows kc*128+p.

Device per batch:
  1. Logits A = Q K^T per k-quarter (512 keys), fp16 matmuls, fp32 PSUM.
     Online softmax: per-quarter negated max m_q, e = exp(a - m_q) fp16,
     accumulated sums.
  2. Merge: global max, f_q = exp(m_q - m) rescale of E, r = 1/sum.
  3. E rows xbar-transposed per q-tile, cast fp8.
  4. C = E8^T.T @ (K8 + Klo8): fp8 DoubleRow matmuls, each contracting
     (E8*K8 + E8*Klo8) via a stride-0-broadcast lhsT against the
     interleaved hi/lo K planes -- K at ~fp16 accuracy, 2x rate.  Four
     passes over d-quarters with double-buffered K tiles in the slots Q
     vacates after the logits phase.  Normalization by r folds into the
     PSUM->SBUF copy (ACT scale); output stored fp16.

Measured end-to-end relative error ~0.004 (gate 2e-2).
"""

import numpy as np
import ml_dtypes

import concourse.bass as bass
import concourse.bacc as bacc
import concourse.mybir as mybir
import concourse.tile as tile
from concourse.bass_utils import run_bass_kernel_spmd

P = 128
N_CORES = 8
B_FULL, LQ, LK, D = 16, 1024, 2048, 4096
B_PER_CORE = B_FULL // N_CORES  # 2

F16 = mybir.dt.float16
F32 = mybir.dt.float32
F8 = mybir.dt.float8e4
AX = mybir.AxisListType
AF = mybir.ActivationFunctionType
DR = mybir.MatmulPerfMode.DoubleRow

E4M3 = ml_dtypes.float8_e4m3


def build_program(b_per_core=B_PER_CORE, lq=LQ, lk=LK, d=D):
    nqt = lq // P          # 8 q-tiles
    nkc = lk // P          # 16 k-chunks
    nqtr = 4               # k-quarters for online softmax
    qtr_k = lk // nqtr     # 512 keys per quarter
    dc_n = d // P          # 32 d-chunks
    nqd = 4                # d-quarters for the second matmul
    qd_d = d // nqd        # 1024

    nc = bacc.Bacc("TRN2", target_bir_lowering=False, debug=False, num_swdge_queues=4)
    # Q/K^T as fp8 hi/lo plane pairs (same bytes as fp16): logits are
    # computed as (Qh+Ql)@Kh + Qh@Kl via DoubleRow, dropping only Ql@Kl.
    # Q is q-tile-major so per-q-tile loads stay contiguous.
    qt_dram = nc.dram_tensor(
        "qt8", [b_per_core, P, lq // P, dc_n, 2, P], F8, kind="ExternalInput"
    ).ap()
    kt_dram = nc.dram_tensor(
        "kt8", [b_per_core, P, dc_n, 2, lk], F8, kind="ExternalInput"
    ).ap()
    kn_dram = nc.dram_tensor("kn8", [b_per_core, P, nkc, 2, d], F8, kind="ExternalInput").ap()
    o_dram = nc.dram_tensor("out", [b_per_core, lq, d], F16, kind="ExternalOutput").ap()

    with tile.TileContext(nc) as tc:
        with (
            # 2x 32KB: Q lo/hi halves during logits, then K8/Klo8 d-quarters
            tc.tile_pool(name="qkn", bufs=2) as qkn,
            # 2x 32KB: K^T quarters (rotating)
            tc.tile_pool(name="ktq", bufs=2) as ktq_pool,
            # 32KB: unscaled/rescaled E [qt, k] fp16
            tc.tile_pool(name="epool", bufs=1) as epool,
            # 16KB: E^T fp8 for the whole batch [qt, kc, q]
            tc.tile_pool(name="e8t", bufs=8) as e8tp,
            # 4KB: fp16 E^T staging per q-tile
            tc.tile_pool(name="ett", bufs=2) as ettp,
            # 2KB: fp16 output staging
            tc.tile_pool(name="cout", bufs=6) as coutp,
            tc.tile_pool(name="stats", bufs=2) as stats,
            tc.tile_pool(name="psumL", bufs=2, space="PSUM") as psumL,
            tc.tile_pool(name="psumC", bufs=3, space="PSUM") as psumC,
        ):
            for b in range(b_per_core):
                # ---- loads for this batch (SP queue, slot waits pace them) ----
                ktq = []
                k0 = ktq_pool.tile([P, dc_n, 2, qtr_k], F8, tag="ktq", name=f"ktq_{b}_0")
                q_lo = qkn.tile([P, 4, dc_n, 2, P], F8, tag="qkn", name=f"qlo_{b}")
                q_hi = qkn.tile([P, 4, dc_n, 2, P], F8, tag="qkn", name=f"qhi_{b}")
                ktq.append(k0)
                if b == 0:
                    # cold start: interleave fine K^T-quarter-0 / Q pieces so
                    # the first logits matmuls start after ~2 small transfers
                    # qt0's group needs ALL of k0 but only q-tile 0: finish k0
                    # before q-tiles 1-3 so the first group completes ASAP
                    k_pieces = [(0, 4), (4, 12), (12, 20), (20, 28), (28, 32)]
                    for i, kp in enumerate(k_pieces):
                        nc.sync.dma_start(
                            out=k0[:, kp[0] : kp[1], :, :],
                            in_=kt_dram[b, :, kp[0] : kp[1], :, 0:qtr_k],
                        )
                        if i == 0:
                            nc.sync.dma_start(
                                out=q_lo[:, 0, :, :, :],
                                in_=qt_dram[b, :, 0, :, :, :],
                            )
                    for qp in (1, 2, 3):
                        nc.sync.dma_start(
                            out=q_lo[:, qp, :, :, :],
                            in_=qt_dram[b, :, qp, :, :, :],
                        )
                if b != 0:
                    nc.sync.dma_start(out=k0[:], in_=kt_dram[b, :, :, :, 0:qtr_k])
                    nc.sync.dma_start(out=q_lo[:], in_=qt_dram[b, :, 0:4, :, :, :])
                # q_hi is needed a quarter-length before K^T quarter 1
                nc.sync.dma_start(out=q_hi[:], in_=qt_dram[b, :, 4:8, :, :, :])
                k1 = ktq_pool.tile([P, dc_n, 2, qtr_k], F8, tag="ktq", name=f"ktq_{b}_1")
                nc.sync.dma_start(out=k1[:], in_=kt_dram[b, :, :, :, qtr_k : 2 * qtr_k])
                ktq.append(k1)

                for q4 in (2, 3):
                    kq = ktq_pool.tile([P, dc_n, 2, qtr_k], F8, tag="ktq",
                                       name=f"ktq_{b}_{q4}")
                    nc.sync.dma_start(
                        out=kq[:], in_=kt_dram[b, :, :, :, q4 * qtr_k : (q4 + 1) * qtr_k]
                    )
                    ktq.append(kq)

                def q_tile(qt):
                    return (q_lo if qt < 4 else q_hi), qt % 4

                # ---- per-batch softmax stats ----
                M = stats.tile([P, nqt, nqtr], F32, tag="m", name=f"M_{b}")
                S = stats.tile([P, nqt, nqtr], F32, tag="s", name=f"S_{b}")
                F = stats.tile([P, nqt, nqtr], F32, tag="f", name=f"F_{b}")
                R = stats.tile([P, nqt], F32, tag="r", name=f"R_{b}")
                E = epool.tile([P, nqt, lk], F16, tag="e", name=f"E_{b}")

                # ---- logits + per-quarter online softmax ----
                # During the last quarter each finished q-tile immediately
                # runs its merge + E^T transpose + fp8 cast, and the first
                # K8/Klo8 d-quarter loads slot in between, so the second
                # matmul starts with zero PE idle.
                e8t_tiles = {}
                knq_tiles = {}

                def merge_chain(qt, aps):
                    """Runs after quarter 3's reduce_max.  Quarter 3's exp uses
                    the GLOBAL max as bias (known now), so its E slice needs no
                    rescale and quarters 0-2 rescale in parallel with the exp —
                    the E^T transpose launches right after both finish."""
                    negm = stats.tile([P, 1], F32, tag="negm", name=f"negm_{b}_{qt}")
                    nc.vector.tensor_reduce(
                        negm, M[:, qt, :], axis=AX.X, op=mybir.AluOpType.min
                    )
                    nc.scalar.activation(
                        F[:, qt, :], M[:, qt, :], AF.Exp, bias=negm, scale=-1.0
                    )
                    for q4 in range(nqtr - 1):
                        sl = E[:, qt, q4 * qtr_k : (q4 + 1) * qtr_k]
                        nc.vector.tensor_scalar_mul(sl, sl, F[:, qt, q4 : q4 + 1])
                    q3 = nqtr - 1
                    nc.scalar.activation(
                        E[:, qt, q3 * qtr_k : (q3 + 1) * qtr_k], aps, AF.Exp,
                        bias=negm, scale=1.0,
                        accum_out=S[:, qt, q3 : q3 + 1],
                    )
                    fs = stats.tile([P, nqtr - 1], F32, tag="fs", name=f"fs_{b}_{qt}")
                    nc.vector.tensor_mul(fs, F[:, qt, : nqtr - 1], S[:, qt, : nqtr - 1])
                    sg = stats.tile([P, 1], F32, tag="sg", name=f"sg_{b}_{qt}")
                    nc.vector.reduce_sum(sg, fs, axis=AX.X)
                    sgt = stats.tile([P, 1], F32, tag="sgt", name=f"sgt_{b}_{qt}")
                    nc.vector.tensor_add(sgt, sg, S[:, qt, q3 : q3 + 1])
                    nc.vector.reciprocal(R[:, qt : qt + 1], sgt)
                    ett = ettp.tile([P, nkc, P], F16, tag="ett", name=f"ett_{b}_{qt}")
                    e8t_tiles[qt] = e8tp.tile([P, nkc, P], F8, tag="e8t",
                                              name=f"e8t_{b}_{qt}")
                    # halves: the second matmul's first k-chunks only wait on
                    # the first half of the transpose+cast chain
                    for h in range(2):
                        hk = nkc // 2
                        nc.sync.dma_start_transpose(
                            ett[:, h * hk : (h + 1) * hk, :],
                            E[:, qt, h * hk * P : (h + 1) * hk * P],
                        )
                        # cast on the otherwise-idle GPSIMD so the DVE FIFO
                        # never waits on the transpose DMA
                        nc.gpsimd.tensor_copy(
                            e8t_tiles[qt][:, h * hk : (h + 1) * hk, :],
                            ett[:, h * hk : (h + 1) * hk, :],
                        )

                def load_knq(qd, pool):
                    knq = pool.tile([P, nkc, 2, qd_d], F8,
                                    tag="ktq" if pool is ktq_pool else "qkn",
                                    name=f"knq_{b}_{qd}")
                    nc.sync.dma_start(
                        out=knq[:],
                        in_=kn_dram[b, :, :, :, qd * qd_d : (qd + 1) * qd_d],
                    )
                    knq_tiles[qd] = knq

                for q4 in range(nqtr):
                    if q4 == nqtr - 1:
                        # K8/Klo8 d-quarter 0 goes in the ktq buffer that
                        # quarter 2 just vacated: its load fully overlaps the
                        # last logits quarter
                        load_knq(0, ktq_pool)
                    for qt in range(nqt):
                        aps = psumL.tile([P, qtr_k], F32, tag="psL",
                                         name=f"aps_{b}_{q4}_{qt}")
                        qtile, qi = q_tile(qt)
                        for dc in range(dc_n):
                            # (Qh+Ql)[dc] @ Kh[dc]
                            nc.tensor.matmul(
                                aps,
                                qtile[:, qi, dc, :, :],
                                ktq[q4][:, dc, 0:1, :].broadcast_to([P, 2, qtr_k]),
                                start=(dc == 0),
                                stop=False,
                                perf_mode=DR,
                            )
                        for dcp in range(dc_n // 2):
                            # Qh[2p]@Kl[2p] + Qh[2p+1]@Kl[2p+1]
                            nc.tensor.matmul(
                                aps,
                                qtile[:, qi, 2 * dcp : 2 * dcp + 2, 0, :],
                                ktq[q4][:, 2 * dcp : 2 * dcp + 2, 1, :],
                                start=False,
                                stop=(dcp == dc_n // 2 - 1),
                                perf_mode=DR,
                            )
                        nc.vector.reduce_max(
                            M[:, qt, q4 : q4 + 1], aps, axis=AX.X, negate=True
                        )
                        if q4 < nqtr - 1:
                            nc.scalar.activation(
                                E[:, qt, q4 * qtr_k : (q4 + 1) * qtr_k], aps, AF.Exp,
                                bias=M[:, qt, q4 : q4 + 1], scale=1.0,
                                accum_out=S[:, qt, q4 : q4 + 1],
                            )
                        else:
                            merge_chain(qt, aps)

                # ---- second matmul: 4 passes over d-quarters ----
                # knq1 is issued only after every E^T transpose so its 11.6us
                # transfer never head-of-line blocks them; it is ready well
                # before pass 1 needs it
                load_knq(1, qkn)
                for qd in range(nqd):
                    if qd not in knq_tiles:
                        load_knq(qd, qkn)
                    knq = knq_tiles[qd]
                    for qt in range(nqt):
                        cps = psumC.tile([P, qd_d], F32, tag="psC",
                                         name=f"cps_{b}_{qd}_{qt}")
                        last_tile = (
                            b == b_per_core - 1 and qd == nqd - 1 and qt == nqt - 1
                        )
                        nbs = qd_d // 512
                        nb_groups = (
                            [[nb] for nb in range(nbs)] if last_tile
                            else [list(range(nbs))]
                        )
                        for grp in nb_groups:
                            for kc in range(nkc):
                                for nb in grp:
                                    nc.tensor.matmul(
                                        cps[:, nb * 512 : (nb + 1) * 512],
                                        e8t_tiles[qt][:, kc : kc + 1, :].broadcast_to([P, 2, P]),
                                        knq[:, kc, :, nb * 512 : (nb + 1) * 512],
                                        start=(kc == 0),
                                        stop=(kc == nkc - 1),
                                        perf_mode=DR,
                                    )
                            if last_tile:
                                # drain the tail in halves so the final store
                                # overlaps the last accumulation group
                                for nb in grp:
                                    c_out = coutp.tile([P, 512], F16, tag="co",
                                                       name=f"co_l_{nb}")
                                    nc.scalar.mul(
                                        c_out, cps[:, nb * 512 : (nb + 1) * 512],
                                        R[:, qt : qt + 1],
                                    )
                                    nc.scalar.dma_start(
                                        out=o_dram[
                                            b, qt * P : (qt + 1) * P,
                                            qd * qd_d + nb * 512 : qd * qd_d + (nb + 1) * 512,
                                        ],
                                        in_=c_out,
                                    )
                        if not last_tile:
                            c_out = coutp.tile([P, qd_d], F16, tag="co",
                                               name=f"co_{b}_{qd}_{qt}")
                            nc.scalar.mul(c_out, cps, R[:, qt : qt + 1])
                            # store right behind the copy on the ACT queue;
                            # Pool stays dedicated to the fp8 casts
                            nc.scalar.dma_start(
                                out=o_dram[b, qt * P : (qt + 1) * P, qd * qd_d : (qd + 1) * qd_d],
                                in_=c_out,
                            )
    nc.compile()
    return nc


_PROGRAM = None


def _get_program():
    global _PROGRAM
    if _PROGRAM is None:
        _PROGRAM = build_program()
    return _PROGRAM


LAST_RESULTS = None  # BassKernelResults of the most recent kernel() call


def _hilo(x: np.ndarray):
    hi = x.astype(E4M3)
    lo = (x - hi.astype(np.float32)).astype(E4M3)
    return np.asarray(hi), np.asarray(lo)


def _prep_core(qb: np.ndarray, kb: np.ndarray):
    """Host-side layout prep for one core's batch slice (see module doc)."""
    b = qb.shape[0]
    qh, ql = _hilo(qb)
    kh, kl = _hilo(kb)
    # qt8 [b, p, qt, dc, 2, qq]: plane j of (Qh, Ql) at [.., j, qq]
    # from Q[b, qt*128+qq, dc*128+p]
    qt8 = np.ascontiguousarray(
        np.stack(
            [
                qh.reshape(b, LQ // P, P, D // P, P),
                ql.reshape(b, LQ // P, P, D // P, P),
            ],
            axis=5,
        ).transpose(0, 4, 1, 3, 5, 2)
    )
    # kt8 [b, p, dc, 2, k] from K[b, k, dc*128+p]
    kt8 = np.ascontiguousarray(
        np.stack(
            [
                kh.reshape(b, LK, D // P, P),
                kl.reshape(b, LK, D // P, P),
            ],
            axis=4,
        ).transpose(0, 3, 2, 4, 1)
    )
    # kn8 [b, p, kc, 2, d]: natural-layout (K8, Klo8) planes for matmul 2
    kn8 = np.ascontiguousarray(
        np.stack(
            [
                kh.reshape(b, LK // P, P, D),
                kl.reshape(b, LK // P, P, D),
            ],
            axis=3,
        ).transpose(0, 2, 1, 3, 4)
    )
    return {"qt8": qt8, "kt8": kt8, "kn8": kn8}


def kernel(query: np.ndarray, key: np.ndarray) -> np.ndarray:
    global LAST_RESULTS
    query = np.ascontiguousarray(query, dtype=np.float32)
    key = np.ascontiguousarray(key, dtype=np.float32)
    assert query.shape == (B_FULL, LQ, D), query.shape
    assert key.shape == (B_FULL, LK, D), key.shape

    nc = _get_program()
    in_maps = [
        _prep_core(
            query[i * B_PER_CORE : (i + 1) * B_PER_CORE],
            key[i * B_PER_CORE : (i + 1) * B_PER_CORE],
        )
        for i in range(N_CORES)
    ]
    try:
        res = run_bass_kernel_spmd(nc, in_maps, core_ids=list(range(N_CORES)))
    except Exception:
        # one retry: absorbs transient device wedges (NRT_EXEC_UNIT_*)
        res = run_bass_kernel_spmd(nc, in_maps, core_ids=list(range(N_CORES)))
    LAST_RESULTS = res
    out = np.concatenate([r["out"] for r in res.results], axis=0)
    return np.ascontiguousarray(out.astype(np.float32))


# revision 37
# speedup vs baseline: 1.1733x; 1.0269x over previous
"""DotAttention Trainium2 Bass kernel.

out[b] = softmax(Q[b] @ K[b]^T, axis=-1) @ K[b]
  Q: [16, 1024, 4096] f32, K: [16, 2048, 4096] f32 -> out [16, 1024, 4096] f32

Sharding: batch dim across 8 NeuronCores (2 batches/core), fully local.

Host-side prep (inside kernel(), per core): cast to fp16 and lay the
operands out in matmul-native form so the device does zero transposes or
casts of Q/K:
  - QT  [b, 128p, 1024q, 32dc] fp16 : QT[b,p,q,dc] = Q[b,q,dc*128+p]
  - KT  [b, 128p, 32dc, 2048k] fp16 : KT[b,p,dc,k] = K[b,k,dc*128+p]
  - KN8 [b, 16kc, 128p, 2, 4096] fp8e4m3: plane 0 = fp8(K), plane 1 =
    fp8(K - fp8(K)) (the quantization residual), rows kc*128+p.

Device per batch:
  1. Logits A = Q K^T per k-quarter (512 keys), fp16 matmuls, fp32 PSUM.
     Online softmax: per-quarter negated max m_q, e = exp(a - m_q) fp16,
     accumulated sums.
  2. Merge: global max, f_q = exp(m_q - m) rescale of E, r = 1/sum.
  3. E rows xbar-transposed per q-tile, cast fp8.
  4. C = E8^T.T @ (K8 + Klo8): fp8 DoubleRow matmuls, each contracting
     (E8*K8 + E8*Klo8) via a stride-0-broadcast lhsT against the
     interleaved hi/lo K planes -- K at ~fp16 accuracy, 2x rate.  Four
     passes over d-quarters with double-buffered K tiles in the slots Q
     vacates after the logits phase.  Normalization by r folds into the
     PSUM->SBUF copy (ACT scale); output stored fp16.

Measured end-to-end relative error ~0.004 (gate 2e-2).
"""

import numpy as np
import ml_dtypes

import concourse.bass as bass
import concourse.bacc as bacc
import concourse.mybir as mybir
import concourse.tile as tile
from concourse.bass_utils import run_bass_kernel_spmd

P = 128
N_CORES = 8
B_FULL, LQ, LK, D = 16, 1024, 2048, 4096
B_PER_CORE = B_FULL // N_CORES  # 2

F16 = mybir.dt.float16
F32 = mybir.dt.float32
F8 = mybir.dt.float8e4
AX = mybir.AxisListType
AF = mybir.ActivationFunctionType
DR = mybir.MatmulPerfMode.DoubleRow

E4M3 = ml_dtypes.float8_e4m3


def build_program(b_per_core=B_PER_CORE, lq=LQ, lk=LK, d=D):
    nqt = lq // P          # 8 q-tiles
    nkc = lk // P          # 16 k-chunks
    nqtr = 4               # k-quarters for online softmax
    qtr_k = lk // nqtr     # 512 keys per quarter
    dc_n = d // P          # 32 d-chunks
    nqd = 4                # d-quarters for the second matmul
    qd_d = d // nqd        # 1024

    nc = bacc.Bacc("TRN2", target_bir_lowering=False, debug=False, num_swdge_queues=4)
    # Q/K^T as fp8 hi/lo plane pairs (same bytes as fp16): logits are
    # computed as (Qh+Ql)@Kh + Qh@Kl via DoubleRow, dropping only Ql@Kl.
    # Q is q-tile-major so per-q-tile loads stay contiguous.
    qt_dram = nc.dram_tensor(
        "qt8", [b_per_core, P, lq // P, dc_n, 2, P], F8, kind="ExternalInput"
    ).ap()
    kt_dram = nc.dram_tensor(
        "kt8", [b_per_core, P, dc_n, 2, lk], F8, kind="ExternalInput"
    ).ap()
    kn_dram = nc.dram_tensor("kn8", [b_per_core, P, nkc, 2, d], F8, kind="ExternalInput").ap()
    o_dram = nc.dram_tensor("out", [b_per_core, lq, d], F16, kind="ExternalOutput").ap()

    with tile.TileContext(nc) as tc:
        with (
            # 2x 32KB: Q lo/hi halves during logits, then K8/Klo8 d-quarters
            tc.tile_pool(name="qkn", bufs=2) as qkn,
            # 2x 32KB: K^T quarters (rotating)
            tc.tile_pool(name="ktq", bufs=2) as ktq_pool,
            # 32KB: unscaled/rescaled E [qt, k] fp16
            tc.tile_pool(name="epool", bufs=1) as epool,
            # 16KB: E^T fp8 for the whole batch [qt, kc, q]
            tc.tile_pool(name="e8t", bufs=8) as e8tp,
            # 4KB: fp16 E^T staging per q-tile
            tc.tile_pool(name="ett", bufs=3) as ettp,
            # 2KB: fp16 output staging
            tc.tile_pool(name="cout", bufs=6) as coutp,
            tc.tile_pool(name="stats", bufs=2) as stats,
            tc.tile_pool(name="psumL", bufs=2, space="PSUM") as psumL,
            tc.tile_pool(name="psumC", bufs=3, space="PSUM") as psumC,
        ):
            for b in range(b_per_core):
                # ---- loads for this batch (SP queue, slot waits pace them) ----
                ktq = []
                k0 = ktq_pool.tile([P, dc_n, 2, qtr_k], F8, tag="ktq", name=f"ktq_{b}_0")
                q_lo = qkn.tile([P, 4, dc_n, 2, P], F8, tag="qkn", name=f"qlo_{b}")
                q_hi = qkn.tile([P, 4, dc_n, 2, P], F8, tag="qkn", name=f"qhi_{b}")
                ktq.append(k0)
                if b == 0:
                    # cold start: interleave fine K^T-quarter-0 / Q pieces so
                    # the first logits matmuls start after ~2 small transfers
                    # qt0's group needs ALL of k0 but only q-tile 0: finish k0
                    # before q-tiles 1-3 so the first group completes ASAP
                    k_pieces = [(0, 4), (4, 12), (12, 20), (20, 28), (28, 32)]
                    for i, kp in enumerate(k_pieces):
                        nc.sync.dma_start(
                            out=k0[:, kp[0] : kp[1], :, :],
                            in_=kt_dram[b, :, kp[0] : kp[1], :, 0:qtr_k],
                        )
                        if i == 0:
                            nc.sync.dma_start(
                                out=q_lo[:, 0, :, :, :],
                                in_=qt_dram[b, :, 0, :, :, :],
                            )
                    for qp in (1, 2, 3):
                        nc.sync.dma_start(
                            out=q_lo[:, qp, :, :, :],
                            in_=qt_dram[b, :, qp, :, :, :],
                        )
                if b != 0:
                    nc.sync.dma_start(out=k0[:], in_=kt_dram[b, :, :, :, 0:qtr_k])
                    nc.sync.dma_start(out=q_lo[:], in_=qt_dram[b, :, 0:4, :, :, :])
                # q_hi is needed a quarter-length before K^T quarter 1
                nc.sync.dma_start(out=q_hi[:], in_=qt_dram[b, :, 4:8, :, :, :])
                k1 = ktq_pool.tile([P, dc_n, 2, qtr_k], F8, tag="ktq", name=f"ktq_{b}_1")
                nc.sync.dma_start(out=k1[:], in_=kt_dram[b, :, :, :, qtr_k : 2 * qtr_k])
                ktq.append(k1)

                for q4 in (2, 3):
                    kq = ktq_pool.tile([P, dc_n, 2, qtr_k], F8, tag="ktq",
                                       name=f"ktq_{b}_{q4}")
                    nc.sync.dma_start(
                        out=kq[:], in_=kt_dram[b, :, :, :, q4 * qtr_k : (q4 + 1) * qtr_k]
                    )
                    ktq.append(kq)

                def q_tile(qt):
                    return (q_lo if qt < 4 else q_hi), qt % 4

                # ---- per-batch softmax stats ----
                M = stats.tile([P, nqt, nqtr], F32, tag="m", name=f"M_{b}")
                S = stats.tile([P, nqt, nqtr], F32, tag="s", name=f"S_{b}")
                F = stats.tile([P, nqt, nqtr], F32, tag="f", name=f"F_{b}")
                R = stats.tile([P, nqt], F32, tag="r", name=f"R_{b}")
                E = epool.tile([P, nqt, lk], F16, tag="e", name=f"E_{b}")

                # ---- logits + per-quarter online softmax ----
                # During the last quarter each finished q-tile immediately
                # runs its merge + E^T transpose + fp8 cast, and the first
                # K8/Klo8 d-quarter loads slot in between, so the second
                # matmul starts with zero PE idle.
                e8t_tiles = {}
                knq_tiles = {}

                def merge_chain(qt, aps):
                    """Runs after quarter 3's reduce_max.  Quarter 3's exp uses
                    the GLOBAL max as bias (known now), so its E slice needs no
                    rescale and quarters 0-2 rescale in parallel with the exp —
                    the E^T transpose launches right after both finish."""
                    negm = stats.tile([P, 1], F32, tag="negm", name=f"negm_{b}_{qt}")
                    nc.vector.tensor_reduce(
                        negm, M[:, qt, :], axis=AX.X, op=mybir.AluOpType.min
                    )
                    nc.scalar.activation(
                        F[:, qt, :], M[:, qt, :], AF.Exp, bias=negm, scale=-1.0
                    )
                    for q4 in range(nqtr - 1):
                        sl = E[:, qt, q4 * qtr_k : (q4 + 1) * qtr_k]
                        # rescale on ACT directly behind the F-exp: same queue
                        # -> no cross-engine hop before the E^T transpose
                        nc.scalar.mul(sl, sl, F[:, qt, q4 : q4 + 1])
                    q3 = nqtr - 1
                    nc.scalar.activation(
                        E[:, qt, q3 * qtr_k : (q3 + 1) * qtr_k], aps, AF.Exp,
                        bias=negm, scale=1.0,
                        accum_out=S[:, qt, q3 : q3 + 1],
                    )
                    fs = stats.tile([P, nqtr - 1], F32, tag="fs", name=f"fs_{b}_{qt}")
                    nc.vector.tensor_mul(fs, F[:, qt, : nqtr - 1], S[:, qt, : nqtr - 1])
                    sg = stats.tile([P, 1], F32, tag="sg", name=f"sg_{b}_{qt}")
                    nc.vector.reduce_sum(sg, fs, axis=AX.X)
                    sgt = stats.tile([P, 1], F32, tag="sgt", name=f"sgt_{b}_{qt}")
                    nc.vector.tensor_add(sgt, sg, S[:, qt, q3 : q3 + 1])
                    nc.vector.reciprocal(R[:, qt : qt + 1], sgt)
                    ett = ettp.tile([P, nkc, P], F16, tag="ett", name=f"ett_{b}_{qt}")
                    e8t_tiles[qt] = e8tp.tile([P, nkc, P], F8, tag="e8t",
                                              name=f"e8t_{b}_{qt}")
                    # halves: the second matmul's first k-chunks only wait on
                    # the first half of the transpose+cast chain
                    for h in range(2):
                        hk = nkc // 2
                        nc.sync.dma_start_transpose(
                            ett[:, h * hk : (h + 1) * hk, :],
                            E[:, qt, h * hk * P : (h + 1) * hk * P],
                        )
                        # cast on the otherwise-idle GPSIMD so the DVE FIFO
                        # never waits on the transpose DMA
                        nc.gpsimd.tensor_copy(
                            e8t_tiles[qt][:, h * hk : (h + 1) * hk, :],
                            ett[:, h * hk : (h + 1) * hk, :],
                        )

                def load_knq(qd, pool):
                    knq = pool.tile([P, nkc, 2, qd_d], F8,
                                    tag="ktq" if pool is ktq_pool else "qkn",
                                    name=f"knq_{b}_{qd}")
                    nc.sync.dma_start(
                        out=knq[:],
                        in_=kn_dram[b, :, :, :, qd * qd_d : (qd + 1) * qd_d],
                    )
                    knq_tiles[qd] = knq

                for q4 in range(nqtr):
                    if q4 == nqtr - 1:
                        # K8/Klo8 d-quarter 0 goes in the ktq buffer that
                        # quarter 2 just vacated: its load fully overlaps the
                        # last logits quarter
                        load_knq(0, ktq_pool)
                    for qt in range(nqt):
                        aps = psumL.tile([P, qtr_k], F32, tag="psL",
                                         name=f"aps_{b}_{q4}_{qt}")
                        qtile, qi = q_tile(qt)
                        for dc in range(dc_n):
                            # (Qh+Ql)[dc] @ Kh[dc]
                            nc.tensor.matmul(
                                aps,
                                qtile[:, qi, dc, :, :],
                                ktq[q4][:, dc, 0:1, :].broadcast_to([P, 2, qtr_k]),
                                start=(dc == 0),
                                stop=False,
                                perf_mode=DR,
                            )
                        for dcp in range(dc_n // 2):
                            # Qh[2p]@Kl[2p] + Qh[2p+1]@Kl[2p+1]
                            nc.tensor.matmul(
                                aps,
                                qtile[:, qi, 2 * dcp : 2 * dcp + 2, 0, :],
                                ktq[q4][:, 2 * dcp : 2 * dcp + 2, 1, :],
                                start=False,
                                stop=(dcp == dc_n // 2 - 1),
                                perf_mode=DR,
                            )
                        nc.vector.reduce_max(
                            M[:, qt, q4 : q4 + 1], aps, axis=AX.X, negate=True
                        )
                        if q4 < nqtr - 1:
                            nc.scalar.activation(
                                E[:, qt, q4 * qtr_k : (q4 + 1) * qtr_k], aps, AF.Exp,
                                bias=M[:, qt, q4 : q4 + 1], scale=1.0,
                                accum_out=S[:, qt, q4 : q4 + 1],
                            )
                        else:
                            merge_chain(qt, aps)

                # ---- second matmul: 4 passes over d-quarters ----
                # knq1 is issued only after every E^T transpose so its 11.6us
                # transfer never head-of-line blocks them; it is ready well
                # before pass 1 needs it
                load_knq(1, qkn)
                for qd in range(nqd):
                    if qd not in knq_tiles:
                        load_knq(qd, qkn)
                    knq = knq_tiles[qd]
                    for qt in range(nqt):
                        cps = psumC.tile([P, qd_d], F32, tag="psC",
                                         name=f"cps_{b}_{qd}_{qt}")
                        last_tile = (
                            b == b_per_core - 1 and qd == nqd - 1 and qt == nqt - 1
                        )
                        nbs = qd_d // 512
                        nb_groups = (
                            [[nb] for nb in range(nbs)] if last_tile
                            else [list(range(nbs))]
                        )
                        for grp in nb_groups:
                            for kc in range(nkc):
                                for nb in grp:
                                    nc.tensor.matmul(
                                        cps[:, nb * 512 : (nb + 1) * 512],
                                        e8t_tiles[qt][:, kc : kc + 1, :].broadcast_to([P, 2, P]),
                                        knq[:, kc, :, nb * 512 : (nb + 1) * 512],
                                        start=(kc == 0),
                                        stop=(kc == nkc - 1),
                                        perf_mode=DR,
                                    )
                            if last_tile:
                                # drain the tail in halves so the final store
                                # overlaps the last accumulation group
                                for nb in grp:
                                    c_out = coutp.tile([P, 512], F16, tag="co",
                                                       name=f"co_l_{nb}")
                                    nc.scalar.mul(
                                        c_out, cps[:, nb * 512 : (nb + 1) * 512],
                                        R[:, qt : qt + 1],
                                    )
                                    nc.scalar.dma_start(
                                        out=o_dram[
                                            b, qt * P : (qt + 1) * P,
                                            qd * qd_d + nb * 512 : qd * qd_d + (nb + 1) * 512,
                                        ],
                                        in_=c_out,
                                    )
                        if not last_tile:
                            c_out = coutp.tile([P, qd_d], F16, tag="co",
                                               name=f"co_{b}_{qd}_{qt}")
                            nc.scalar.mul(c_out, cps, R[:, qt : qt + 1])
                            # store right behind the copy on the ACT queue;
                            # Pool stays dedicated to the fp8 casts
                            nc.scalar.dma_start(
                                out=o_dram[b, qt * P : (qt + 1) * P, qd * qd_d : (qd + 1) * qd_d],
                                in_=c_out,
                            )
    nc.compile()
    return nc


_PROGRAM = None


def _get_program():
    global _PROGRAM
    if _PROGRAM is None:
        _PROGRAM = build_program()
    return _PROGRAM


LAST_RESULTS = None  # BassKernelResults of the most recent kernel() call


def _hilo(x: np.ndarray):
    hi = x.astype(E4M3)
    lo = (x - hi.astype(np.float32)).astype(E4M3)
    return np.asarray(hi), np.asarray(lo)


def _prep_core(qb: np.ndarray, kb: np.ndarray):
    """Host-side layout prep for one core's batch slice (see module doc)."""
    b = qb.shape[0]
    qh, ql = _hilo(qb)
    kh, kl = _hilo(kb)
    # qt8 [b, p, qt, dc, 2, qq]: plane j of (Qh, Ql) at [.., j, qq]
    # from Q[b, qt*128+qq, dc*128+p]
    qt8 = np.ascontiguousarray(
        np.stack(
            [
                qh.reshape(b, LQ // P, P, D // P, P),
                ql.reshape(b, LQ // P, P, D // P, P),
            ],
            axis=5,
        ).transpose(0, 4, 1, 3, 5, 2)
    )
    # kt8 [b, p, dc, 2, k] from K[b, k, dc*128+p]
    kt8 = np.ascontiguousarray(
        np.stack(
            [
                kh.reshape(b, LK, D // P, P),
                kl.reshape(b, LK, D // P, P),
            ],
            axis=4,
        ).transpose(0, 3, 2, 4, 1)
    )
    # kn8 [b, p, kc, 2, d]: natural-layout (K8, Klo8) planes for matmul 2
    kn8 = np.ascontiguousarray(
        np.stack(
            [
                kh.reshape(b, LK // P, P, D),
                kl.reshape(b, LK // P, P, D),
            ],
            axis=3,
        ).transpose(0, 2, 1, 3, 4)
    )
    return {"qt8": qt8, "kt8": kt8, "kn8": kn8}


def kernel(query: np.ndarray, key: np.ndarray) -> np.ndarray:
    global LAST_RESULTS
    query = np.ascontiguousarray(query, dtype=np.float32)
    key = np.ascontiguousarray(key, dtype=np.float32)
    assert query.shape == (B_FULL, LQ, D), query.shape
    assert key.shape == (B_FULL, LK, D), key.shape

    nc = _get_program()
    in_maps = [
        _prep_core(
            query[i * B_PER_CORE : (i + 1) * B_PER_CORE],
            key[i * B_PER_CORE : (i + 1) * B_PER_CORE],
        )
        for i in range(N_CORES)
    ]
    try:
        res = run_bass_kernel_spmd(nc, in_maps, core_ids=list(range(N_CORES)))
    except Exception:
        # one retry: absorbs transient device wedges (NRT_EXEC_UNIT_*)
        res = run_bass_kernel_spmd(nc, in_maps, core_ids=list(range(N_CORES)))
    LAST_RESULTS = res
    out = np.concatenate([r["out"] for r in res.results], axis=0)
    return np.ascontiguousarray(out.astype(np.float32))


# revision 38
# speedup vs baseline: 1.1777x; 1.0038x over previous
"""DotAttention Trainium2 Bass kernel.

out[b] = softmax(Q[b] @ K[b]^T, axis=-1) @ K[b]
  Q: [16, 1024, 4096] f32, K: [16, 2048, 4096] f32 -> out [16, 1024, 4096] f32

Sharding: batch dim across 8 NeuronCores (2 batches/core), fully local.

Host-side prep (inside kernel(), per core): cast to fp16 and lay the
operands out in matmul-native form so the device does zero transposes or
casts of Q/K:
  - QT  [b, 128p, 1024q, 32dc] fp16 : QT[b,p,q,dc] = Q[b,q,dc*128+p]
  - KT  [b, 128p, 32dc, 2048k] fp16 : KT[b,p,dc,k] = K[b,k,dc*128+p]
  - KN8 [b, 16kc, 128p, 2, 4096] fp8e4m3: plane 0 = fp8(K), plane 1 =
    fp8(K - fp8(K)) (the quantization residual), rows kc*128+p.

Device per batch:
  1. Logits A = Q K^T per k-quarter (512 keys), fp16 matmuls, fp32 PSUM.
     Online softmax: per-quarter negated max m_q, e = exp(a - m_q) fp16,
     accumulated sums.
  2. Merge: global max, f_q = exp(m_q - m) rescale of E, r = 1/sum.
  3. E rows xbar-transposed per q-tile, cast fp8.
  4. C = E8^T.T @ (K8 + Klo8): fp8 DoubleRow matmuls, each contracting
     (E8*K8 + E8*Klo8) via a stride-0-broadcast lhsT against the
     interleaved hi/lo K planes -- K at ~fp16 accuracy, 2x rate.  Four
     passes over d-quarters with double-buffered K tiles in the slots Q
     vacates after the logits phase.  Normalization by r folds into the
     PSUM->SBUF copy (ACT scale); output stored fp16.

Measured end-to-end relative error ~0.004 (gate 2e-2).
"""

import numpy as np
import ml_dtypes

import concourse.bass as bass
import concourse.bacc as bacc
import concourse.mybir as mybir
import concourse.tile as tile
from concourse.bass_utils import run_bass_kernel_spmd

P = 128
N_CORES = 8
B_FULL, LQ, LK, D = 16, 1024, 2048, 4096
B_PER_CORE = B_FULL // N_CORES  # 2

F16 = mybir.dt.float16
F32 = mybir.dt.float32
F8 = mybir.dt.float8e4
AX = mybir.AxisListType
AF = mybir.ActivationFunctionType
DR = mybir.MatmulPerfMode.DoubleRow

E4M3 = ml_dtypes.float8_e4m3


def build_program(b_per_core=B_PER_CORE, lq=LQ, lk=LK, d=D):
    nqt = lq // P          # 8 q-tiles
    nkc = lk // P          # 16 k-chunks
    nqtr = 4               # k-quarters for online softmax
    qtr_k = lk // nqtr     # 512 keys per quarter
    dc_n = d // P          # 32 d-chunks
    nqd = 4                # d-quarters for the second matmul
    qd_d = d // nqd        # 1024

    nc = bacc.Bacc("TRN2", target_bir_lowering=False, debug=False, num_swdge_queues=4)
    # Q/K^T as fp8 hi/lo plane pairs (same bytes as fp16): logits are
    # computed as (Qh+Ql)@Kh + Qh@Kl via DoubleRow, dropping only Ql@Kl.
    # Q is q-tile-major so per-q-tile loads stay contiguous.
    qt_dram = nc.dram_tensor(
        "qt8", [b_per_core, P, lq // P, dc_n, 2, P], F8, kind="ExternalInput"
    ).ap()
    kt_dram = nc.dram_tensor(
        "kt8", [b_per_core, P, dc_n, 2, lk], F8, kind="ExternalInput"
    ).ap()
    kn_dram = nc.dram_tensor("kn8", [b_per_core, P, nkc, 2, d], F8, kind="ExternalInput").ap()
    o_dram = nc.dram_tensor("out", [b_per_core, lq, d], F16, kind="ExternalOutput").ap()

    with tile.TileContext(nc) as tc:
        with (
            # 2x 32KB: Q lo/hi halves during logits, then K8/Klo8 d-quarters
            tc.tile_pool(name="qkn", bufs=2) as qkn,
            # 2x 32KB: K^T quarters (rotating)
            tc.tile_pool(name="ktq", bufs=2) as ktq_pool,
            # 32KB: unscaled/rescaled E [qt, k] fp16
            tc.tile_pool(name="epool", bufs=1) as epool,
            # 16KB: E^T fp8 for the whole batch [qt, kc, q]
            tc.tile_pool(name="e8t", bufs=8) as e8tp,
            # 4KB: fp16 E^T staging per q-tile
            tc.tile_pool(name="ett", bufs=3) as ettp,
            # 2KB: fp16 output staging
            tc.tile_pool(name="cout", bufs=6) as coutp,
            tc.tile_pool(name="stats", bufs=2) as stats,
            tc.tile_pool(name="psumL", bufs=2, space="PSUM") as psumL,
            tc.tile_pool(name="psumC", bufs=3, space="PSUM") as psumC,
        ):
            for b in range(b_per_core):
                # ---- loads for this batch (SP queue, slot waits pace them) ----
                ktq = []
                k0 = ktq_pool.tile([P, dc_n, 2, qtr_k], F8, tag="ktq", name=f"ktq_{b}_0")
                q_lo = qkn.tile([P, 4, dc_n, 2, P], F8, tag="qkn", name=f"qlo_{b}")
                q_hi = qkn.tile([P, 4, dc_n, 2, P], F8, tag="qkn", name=f"qhi_{b}")
                ktq.append(k0)
                if b == 0:
                    # cold start: interleave fine K^T-quarter-0 / Q pieces so
                    # the first logits matmuls start after ~2 small transfers
                    # qt0's group needs ALL of k0 but only q-tile 0: finish k0
                    # before q-tiles 1-3 so the first group completes ASAP
                    k_pieces = [(0, 4), (4, 12), (12, 20), (20, 28), (28, 32)]
                    for i, kp in enumerate(k_pieces):
                        nc.sync.dma_start(
                            out=k0[:, kp[0] : kp[1], :, :],
                            in_=kt_dram[b, :, kp[0] : kp[1], :, 0:qtr_k],
                        )
                        if i == 0:
                            nc.sync.dma_start(
                                out=q_lo[:, 0, :, :, :],
                                in_=qt_dram[b, :, 0, :, :, :],
                            )
                    for qp in (1, 2, 3):
                        nc.sync.dma_start(
                            out=q_lo[:, qp, :, :, :],
                            in_=qt_dram[b, :, qp, :, :, :],
                        )
                if b != 0:
                    nc.sync.dma_start(out=k0[:], in_=kt_dram[b, :, :, :, 0:qtr_k])
                    nc.sync.dma_start(out=q_lo[:], in_=qt_dram[b, :, 0:4, :, :, :])
                # q_hi is needed a quarter-length before K^T quarter 1; at
                # cold start load it per-q-tile so each arrives just in time
                if b == 0:
                    for qp in range(4):
                        nc.sync.dma_start(
                            out=q_hi[:, qp, :, :, :],
                            in_=qt_dram[b, :, 4 + qp, :, :, :],
                        )
                else:
                    nc.sync.dma_start(out=q_hi[:], in_=qt_dram[b, :, 4:8, :, :, :])
                k1 = ktq_pool.tile([P, dc_n, 2, qtr_k], F8, tag="ktq", name=f"ktq_{b}_1")
                nc.sync.dma_start(out=k1[:], in_=kt_dram[b, :, :, :, qtr_k : 2 * qtr_k])
                ktq.append(k1)

                for q4 in (2, 3):
                    kq = ktq_pool.tile([P, dc_n, 2, qtr_k], F8, tag="ktq",
                                       name=f"ktq_{b}_{q4}")
                    nc.sync.dma_start(
                        out=kq[:], in_=kt_dram[b, :, :, :, q4 * qtr_k : (q4 + 1) * qtr_k]
                    )
                    ktq.append(kq)

                def q_tile(qt):
                    return (q_lo if qt < 4 else q_hi), qt % 4

                # ---- per-batch softmax stats ----
                M = stats.tile([P, nqt, nqtr], F32, tag="m", name=f"M_{b}")
                S = stats.tile([P, nqt, nqtr], F32, tag="s", name=f"S_{b}")
                F = stats.tile([P, nqt, nqtr], F32, tag="f", name=f"F_{b}")
                R = stats.tile([P, nqt], F32, tag="r", name=f"R_{b}")
                E = epool.tile([P, nqt, lk], F16, tag="e", name=f"E_{b}")

                # ---- logits + per-quarter online softmax ----
                # During the last quarter each finished q-tile immediately
                # runs its merge + E^T transpose + fp8 cast, and the first
                # K8/Klo8 d-quarter loads slot in between, so the second
                # matmul starts with zero PE idle.
                e8t_tiles = {}
                knq_tiles = {}

                def merge_chain(qt, aps):
                    """Runs after quarter 3's reduce_max.  Quarter 3's exp uses
                    the GLOBAL max as bias (known now), so its E slice needs no
                    rescale and quarters 0-2 rescale in parallel with the exp —
                    the E^T transpose launches right after both finish."""
                    negm = stats.tile([P, 1], F32, tag="negm", name=f"negm_{b}_{qt}")
                    nc.vector.tensor_reduce(
                        negm, M[:, qt, :], axis=AX.X, op=mybir.AluOpType.min
                    )
                    nc.scalar.activation(
                        F[:, qt, :], M[:, qt, :], AF.Exp, bias=negm, scale=-1.0
                    )
                    for q4 in range(nqtr - 1):
                        sl = E[:, qt, q4 * qtr_k : (q4 + 1) * qtr_k]
                        # rescale on ACT directly behind the F-exp: same queue
                        # -> no cross-engine hop before the E^T transpose
                        nc.scalar.mul(sl, sl, F[:, qt, q4 : q4 + 1])
                    q3 = nqtr - 1
                    nc.scalar.activation(
                        E[:, qt, q3 * qtr_k : (q3 + 1) * qtr_k], aps, AF.Exp,
                        bias=negm, scale=1.0,
                        accum_out=S[:, qt, q3 : q3 + 1],
                    )
                    fs = stats.tile([P, nqtr - 1], F32, tag="fs", name=f"fs_{b}_{qt}")
                    nc.vector.tensor_mul(fs, F[:, qt, : nqtr - 1], S[:, qt, : nqtr - 1])
                    sg = stats.tile([P, 1], F32, tag="sg", name=f"sg_{b}_{qt}")
                    nc.vector.reduce_sum(sg, fs, axis=AX.X)
                    sgt = stats.tile([P, 1], F32, tag="sgt", name=f"sgt_{b}_{qt}")
                    nc.vector.tensor_add(sgt, sg, S[:, qt, q3 : q3 + 1])
                    nc.vector.reciprocal(R[:, qt : qt + 1], sgt)
                    ett = ettp.tile([P, nkc, P], F16, tag="ett", name=f"ett_{b}_{qt}")
                    e8t_tiles[qt] = e8tp.tile([P, nkc, P], F8, tag="e8t",
                                              name=f"e8t_{b}_{qt}")
                    # halves: the second matmul's first k-chunks only wait on
                    # the first half of the transpose+cast chain
                    for h in range(2):
                        hk = nkc // 2
                        nc.sync.dma_start_transpose(
                            ett[:, h * hk : (h + 1) * hk, :],
                            E[:, qt, h * hk * P : (h + 1) * hk * P],
                        )
                        # cast on the otherwise-idle GPSIMD so the DVE FIFO
                        # never waits on the transpose DMA
                        nc.gpsimd.tensor_copy(
                            e8t_tiles[qt][:, h * hk : (h + 1) * hk, :],
                            ett[:, h * hk : (h + 1) * hk, :],
                        )

                def load_knq(qd, pool):
                    knq = pool.tile([P, nkc, 2, qd_d], F8,
                                    tag="ktq" if pool is ktq_pool else "qkn",
                                    name=f"knq_{b}_{qd}")
                    nc.sync.dma_start(
                        out=knq[:],
                        in_=kn_dram[b, :, :, :, qd * qd_d : (qd + 1) * qd_d],
                    )
                    knq_tiles[qd] = knq

                for q4 in range(nqtr):
                    if q4 == nqtr - 1:
                        # K8/Klo8 d-quarter 0 goes in the ktq buffer that
                        # quarter 2 just vacated: its load fully overlaps the
                        # last logits quarter
                        load_knq(0, ktq_pool)
                    for qt in range(nqt):
                        aps = psumL.tile([P, qtr_k], F32, tag="psL",
                                         name=f"aps_{b}_{q4}_{qt}")
                        qtile, qi = q_tile(qt)
                        for dc in range(dc_n):
                            # (Qh+Ql)[dc] @ Kh[dc]
                            nc.tensor.matmul(
                                aps,
                                qtile[:, qi, dc, :, :],
                                ktq[q4][:, dc, 0:1, :].broadcast_to([P, 2, qtr_k]),
                                start=(dc == 0),
                                stop=False,
                                perf_mode=DR,
                            )
                        for dcp in range(dc_n // 2):
                            # Qh[2p]@Kl[2p] + Qh[2p+1]@Kl[2p+1]
                            nc.tensor.matmul(
                                aps,
                                qtile[:, qi, 2 * dcp : 2 * dcp + 2, 0, :],
                                ktq[q4][:, 2 * dcp : 2 * dcp + 2, 1, :],
                                start=False,
                                stop=(dcp == dc_n // 2 - 1),
                                perf_mode=DR,
                            )
                        nc.vector.reduce_max(
                            M[:, qt, q4 : q4 + 1], aps, axis=AX.X, negate=True
                        )
                        if q4 < nqtr - 1:
                            nc.scalar.activation(
                                E[:, qt, q4 * qtr_k : (q4 + 1) * qtr_k], aps, AF.Exp,
                                bias=M[:, qt, q4 : q4 + 1], scale=1.0,
                                accum_out=S[:, qt, q4 : q4 + 1],
                            )
                        else:
                            merge_chain(qt, aps)

                # ---- second matmul: 4 passes over d-quarters ----
                # knq1 is issued only after every E^T transpose so its 11.6us
                # transfer never head-of-line blocks them; it is ready well
                # before pass 1 needs it
                load_knq(1, qkn)
                for qd in range(nqd):
                    if qd not in knq_tiles:
                        load_knq(qd, qkn)
                    knq = knq_tiles[qd]
                    for qt in range(nqt):
                        cps = psumC.tile([P, qd_d], F32, tag="psC",
                                         name=f"cps_{b}_{qd}_{qt}")
                        last_tile = (
                            b == b_per_core - 1 and qd == nqd - 1 and qt == nqt - 1
                        )
                        nbs = qd_d // 512
                        nb_groups = (
                            [[nb] for nb in range(nbs)] if last_tile
                            else [list(range(nbs))]
                        )
                        for grp in nb_groups:
                            for kc in range(nkc):
                                for nb in grp:
                                    nc.tensor.matmul(
                                        cps[:, nb * 512 : (nb + 1) * 512],
                                        e8t_tiles[qt][:, kc : kc + 1, :].broadcast_to([P, 2, P]),
                                        knq[:, kc, :, nb * 512 : (nb + 1) * 512],
                                        start=(kc == 0),
                                        stop=(kc == nkc - 1),
                                        perf_mode=DR,
                                    )
                            if last_tile:
                                # drain the tail in halves so the final store
                                # overlaps the last accumulation group
                                for nb in grp:
                                    c_out = coutp.tile([P, 512], F16, tag="co",
                                                       name=f"co_l_{nb}")
                                    nc.scalar.mul(
                                        c_out, cps[:, nb * 512 : (nb + 1) * 512],
                                        R[:, qt : qt + 1],
                                    )
                                    nc.scalar.dma_start(
                                        out=o_dram[
                                            b, qt * P : (qt + 1) * P,
                                            qd * qd_d + nb * 512 : qd * qd_d + (nb + 1) * 512,
                                        ],
                                        in_=c_out,
                                    )
                        if not last_tile:
                            c_out = coutp.tile([P, qd_d], F16, tag="co",
                                               name=f"co_{b}_{qd}_{qt}")
                            nc.scalar.mul(c_out, cps, R[:, qt : qt + 1])
                            # store right behind the copy on the ACT queue;
                            # Pool stays dedicated to the fp8 casts
                            nc.scalar.dma_start(
                                out=o_dram[b, qt * P : (qt + 1) * P, qd * qd_d : (qd + 1) * qd_d],
                                in_=c_out,
                            )
    nc.compile()
    return nc


_PROGRAM = None


def _get_program():
    global _PROGRAM
    if _PROGRAM is None:
        _PROGRAM = build_program()
    return _PROGRAM


LAST_RESULTS = None  # BassKernelResults of the most recent kernel() call


def _hilo(x: np.ndarray):
    hi = x.astype(E4M3)
    lo = (x - hi.astype(np.float32)).astype(E4M3)
    return np.asarray(hi), np.asarray(lo)


def _prep_core(qb: np.ndarray, kb: np.ndarray):
    """Host-side layout prep for one core's batch slice (see module doc)."""
    b = qb.shape[0]
    qh, ql = _hilo(qb)
    kh, kl = _hilo(kb)
    # qt8 [b, p, qt, dc, 2, qq]: plane j of (Qh, Ql) at [.., j, qq]
    # from Q[b, qt*128+qq, dc*128+p]
    qt8 = np.ascontiguousarray(
        np.stack(
            [
                qh.reshape(b, LQ // P, P, D // P, P),
                ql.reshape(b, LQ // P, P, D // P, P),
            ],
            axis=5,
        ).transpose(0, 4, 1, 3, 5, 2)
    )
    # kt8 [b, p, dc, 2, k] from K[b, k, dc*128+p]
    kt8 = np.ascontiguousarray(
        np.stack(
            [
                kh.reshape(b, LK, D // P, P),
                kl.reshape(b, LK, D // P, P),
            ],
            axis=4,
        ).transpose(0, 3, 2, 4, 1)
    )
    # kn8 [b, p, kc, 2, d]: natural-layout (K8, Klo8) planes for matmul 2
    kn8 = np.ascontiguousarray(
        np.stack(
            [
                kh.reshape(b, LK // P, P, D),
                kl.reshape(b, LK // P, P, D),
            ],
            axis=3,
        ).transpose(0, 2, 1, 3, 4)
    )
    return {"qt8": qt8, "kt8": kt8, "kn8": kn8}


def kernel(query: np.ndarray, key: np.ndarray) -> np.ndarray:
    global LAST_RESULTS
    query = np.ascontiguousarray(query, dtype=np.float32)
    key = np.ascontiguousarray(key, dtype=np.float32)
    assert query.shape == (B_FULL, LQ, D), query.shape
    assert key.shape == (B_FULL, LK, D), key.shape

    nc = _get_program()
    in_maps = [
        _prep_core(
            query[i * B_PER_CORE : (i + 1) * B_PER_CORE],
            key[i * B_PER_CORE : (i + 1) * B_PER_CORE],
        )
        for i in range(N_CORES)
    ]
    try:
        res = run_bass_kernel_spmd(nc, in_maps, core_ids=list(range(N_CORES)))
    except Exception:
        # one retry: absorbs transient device wedges (NRT_EXEC_UNIT_*)
        res = run_bass_kernel_spmd(nc, in_maps, core_ids=list(range(N_CORES)))
    LAST_RESULTS = res
    out = np.concatenate([r["out"] for r in res.results], axis=0)
    return np.ascontiguousarray(out.astype(np.float32))
